# revision 5
# baseline (speedup 1.0000x reference)
"""LoRA linear layer on 8 Trainium2 NeuronCores — bf16 + quarter-fp8 DoubleRow.

Computes y = x @ W^T + b + 2.0 * (x @ A^T) @ B^T for
x:[4,4096,1024], W:[1024,1024], b:[1024], A:[16,1024], B:[1024,16].

Host folds the LoRA update into the weight (W_eff = W + 2*B@A, exact), so the
device kernel is a single GEMM + bias, data-parallel over tokens (2048/core).

v11 = v8 + bias on the sync ring (was ready only at 24.4us behind the W queue, gating all evictions) (v7's 6-wide ko sweep outpaced 2-ko granule supply, 3.7us of PE stalls): for steady-state super-chunks, the first
256 of the 1024 contraction elements run as ONE DoubleRow fp8e4 matmul
(2 fp8/PE-cell, 0.5 cyc/row) into a separate PSUM bank, descaled by 1/4096 on
the Scalar engine during eviction; the remaining 768 elements stay bf16.
Measured-numerics rel-err: 0.0144 (gate 2e-2). The first super-chunk and the
very last m-tile stay pure bf16 (protects the DMA/clock ramp and the tail).
Per half-group: 6x216ns bf16 MMs + ~110ns DR MM vs 8x216ns -> ~0.3us saved
x 22 half-groups.

Scales (powers of 2, exact): x8 = e4m3(16*x), w8 = e4m3(256*W_eff) for
d in [0,256); PSUM_fp8 = 4096*(x@W)_[0:256), combined as
y = P_bf16 + P_fp8/4096 + bias.
"""

import ml_dtypes
import numpy as np

import concourse.mybir as mybir
import concourse.tile as tile
from concourse import bacc
from concourse.bass_utils import run_bass_kernel_spmd

N_CORES = 8
P = 128
D = 1024  # in_features (contraction)
O = 1024  # out_features
M_TOTAL = 4 * 4096  # tokens
M = M_TOTAL // N_CORES  # tokens per core
KO = D // P  # k-subtiles (8); ko 0-1 ride the fp8 path in steady state
SC = 512  # m super-chunk
SCALING = 2.0
SX = 16.0  # fp8 x scale
SW = 256.0  # fp8 W scale

TRACE = False
LAST_RESULT = None

_NC_CACHE = None


def _build_nc():
    f32 = mybir.dt.float32
    bf16 = mybir.dt.bfloat16
    f8 = mybir.dt.float8e4

    nc = bacc.Bacc("TRN2", debug=False)
    xT = nc.dram_tensor("xT", [D, M], bf16, kind="ExternalInput")
    wT = nc.dram_tensor("wT", [D, O], bf16, kind="ExternalInput")
    x8 = nc.dram_tensor("x8", [2 * P, M], f8, kind="ExternalInput")
    w8 = nc.dram_tensor("w8", [2 * P, O], f8, kind="ExternalInput")
    bias = nc.dram_tensor("bias", [P, O], f32, kind="ExternalInput")
    y = nc.dram_tensor("y", [M, O], bf16, kind="ExternalOutput")

    xT_v = xT[:].rearrange("(ko p) m -> p ko m", p=P)  # [128, 8, 2048]
    wT_v = wT[:].rearrange("(ko p) o -> p ko o", p=P)  # [128, 8, 1024]
    x8_v = x8[:].rearrange("(j p) m -> p j m", p=P)  # [128, 2, 2048]
    w8_v = w8[:].rearrange("(j p) o -> p j o", p=P)  # [128, 2, 1024]
    y_v = y[:].rearrange("(mt p) o -> p mt o", p=P)  # [128, 16, 1024]

    n_sc = M // SC
    MPC = SC // P  # m-tiles per super-chunk (4)
    with tile.TileContext(nc) as tc:
        with (
            tc.tile_pool(name="wpool", bufs=1) as wpool,
            tc.tile_pool(name="bpool", bufs=1) as bpool,
            tc.tile_pool(name="xpool", bufs=3) as xpool,
            tc.tile_pool(name="opool", bufs=6) as opool,
            tc.tile_pool(name="tpool", bufs=4) as tpool,
            tc.tile_pool(name="psum", bufs=6, space="PSUM") as psum,
        ):
            # Zero warmup tile on the Vector engine; N=512 throwaway matmuls
            # keep PE duty high so the HAM clock-gate flips one 3.4us window
            # after the first LDWEIGHTS while the first x/W slices stream in.
            zt = wpool.tile([P, 512], bf16, tag="warm")
            nc.vector.memset(zt[:], 0.0)
            wps = psum.tile([P, 512], mybir.dt.float32, tag="ps", name="wps")
            for _ in range(8):
                nc.tensor.matmul(wps[:], zt[:, :P], zt[:], start=True, stop=True)

            # sc0 x + W on separate HWDGE rings, 1-ko granules for the first
            # two k-subtiles then 2-ko granules.
            GRAN = [(k, k + 1) for k in range(KO)]
            x0 = [None] * KO
            wt = [None] * KO
            for lo, hi in GRAN:
                t = xpool.tile([P, hi - lo, SC], bf16, tag="x0", name=f"x0_{lo}", bufs=8)
                nc.sync.dma_start(t[:], xT_v[:, lo:hi, 0:SC])
                w = wpool.tile([P, hi - lo, O], bf16, tag=f"w{lo}")
                nc.scalar.dma_start(w[:], wT_v[:, lo:hi, :])
                for ko in range(lo, hi):
                    x0[ko] = (t, ko - lo)
                    wt[ko] = (w, ko - lo)
            # bias gates every eviction; the scalar ring is congested with
            # 2 MiB of W singles, so ship bias on the lighter sync ring
            bt = bpool.tile([P, O], f32)
            nc.sync.dma_start(bt[:], bias[:])
            w8t = wpool.tile([P, 2, O], f8, tag="w8")
            nc.scalar.dma_start(w8t[:], w8_v[:])

            xts = {}

            def load_x(sc):
                # steady-state super-chunks only need bf16 for ko 2..7
                t = xpool.tile([P, KO - 2, SC], bf16, tag="xsc", name=f"x{sc}")
                nc.sync.dma_start(t[:], xT_v[:, 2:KO, sc * SC : (sc + 1) * SC])
                xts[sc] = t

            def x_slice(sc, ko, mt_i):
                lo = mt_i * P
                if sc == 0:
                    t, j = x0[ko]
                    return t[:, j, lo : lo + P]
                return xts[sc][:, ko - 2, lo : lo + P]

            def w_slice(ko, half):
                w, j = wt[ko]
                return w[:, j, half * 512 : (half + 1) * 512]

            def evict_half(ps, ot, half):
                nc.vector.tensor_tensor(
                    ot[:, half * 512 : (half + 1) * 512],
                    ps[:],
                    bt[:, half * 512 : (half + 1) * 512],
                    mybir.AluOpType.add,
                )

            # --- sc0: pure bf16, ko-outer over mt0-2 (6 PSUM banks), then mt3 ---
            load_x(1)
            pss = [
                [
                    psum.tile([P, 512], mybir.dt.float32, tag="ps", name=f"ps0_{i}_{h}")
                    for h in range(2)
                ]
                for i in range(3)
            ]
            ots = [
                opool.tile([P, O], bf16, tag="ot", name=f"ot0_{i}") for i in range(3)
            ]
            for ko in range(KO):
                last = ko == KO - 1
                for mt_i in range(3):
                    for half in range(2):
                        nc.tensor.matmul(
                            pss[mt_i][half][:],
                            x_slice(0, ko, mt_i),
                            w_slice(ko, half),
                            start=ko == 0,
                            stop=last,
                        )
                    if last:
                        for half in range(2):
                            evict_half(pss[mt_i][half], ots[mt_i], half)
                        nc.scalar.dma_start(y_v[:, mt_i, :], ots[mt_i][:])
            ot3 = opool.tile([P, O], bf16, tag="ot", name="ot0_3")
            for half in range(2):
                ps = psum.tile([P, 512], mybir.dt.float32, tag="ps", name=f"ps0_3_{half}")
                for ko in range(KO):
                    nc.tensor.matmul(
                        ps[:],
                        x_slice(0, ko, 3),
                        w_slice(ko, half),
                        start=ko == 0,
                        stop=ko == KO - 1,
                    )
                evict_half(ps, ot3, half)
            nc.scalar.dma_start(y_v[:, 3, :], ot3[:])
            load_x(2)

            # fp8 x for steady-state tokens (m 512..2048), after sc1's bf16
            x8t = xpool.tile([P, 2, 3 * SC], f8, tag="x8")
            nc.sync.dma_start(x8t[:], x8_v[:, :, SC : 4 * SC])

            def x8_slice(sc, mt_i):
                lo = (sc - 1) * SC + mt_i * P
                return x8t[:, :, lo : lo + P]

            def dr_half(sc, mt_i, ot, half):
                # fp8 DoubleRow: K=256 in one matmul, separate PSUM bank,
                # descale 1/(SX*SW) on the Scalar engine during eviction.
                ps8 = psum.tile(
                    [P, 512], mybir.dt.float32, tag="ps8",
                    name=f"ps8_{sc}_{mt_i}_{half}", bufs=2,
                )
                nc.tensor.matmul(
                    ps8[:],
                    x8_slice(sc, mt_i),
                    w8t[:, :, half * 512 : (half + 1) * 512],
                    start=True,
                    stop=True,
                    perf_mode=mybir.MatmulPerfMode.DoubleRow,
                )
                t8 = tpool.tile([P, 512], f32, tag="t8", name=f"t8_{sc}_{mt_i}_{half}")
                nc.scalar.activation(
                    t8[:], ps8[:], mybir.ActivationFunctionType.Copy,
                    scale=1.0 / (SX * SW),
                )
                hs = slice(half * 512, (half + 1) * 512)
                nc.vector.tensor_tensor(ot[:, hs], ot[:, hs], t8[:], mybir.AluOpType.add)

            # --- sc1..3: mt-outer, bf16 ko2-7 + fp8 DoubleRow for ko0-1;
            # the very last m-tile stays pure bf16 for a short tail ---
            for sc in range(1, n_sc):
                if sc + 2 <= n_sc - 1:
                    load_x(sc + 2)
                for mt_i in range(MPC):
                    mt = sc * MPC + mt_i
                    final = sc == n_sc - 1 and mt_i == MPC - 1
                    ot = opool.tile([P, O], bf16, tag="ot", name=f"ot{sc}_{mt_i}")
                    for half in range(2):
                        ps = psum.tile(
                            [P, 512], mybir.dt.float32, tag="ps",
                            name=f"ps{sc}_{mt_i}_{half}",
                        )
                        if final:
                            # pure bf16: ko 0..7 (ko0-1 from a small late tile)
                            for ko in range(KO):
                                nc.tensor.matmul(
                                    ps[:],
                                    xf[:, ko, :] if ko < 2 else x_slice(sc, ko, mt_i),
                                    w_slice(ko, half),
                                    start=ko == 0,
                                    stop=ko == KO - 1,
                                )
                            evict_half(ps, ot, half)
                            nc.sync.dma_start(
                                y_v[:, mt, half * 512 : (half + 1) * 512],
                                ot[:, half * 512 : (half + 1) * 512],
                            )
                        else:
                            for ko in range(2, KO):
                                nc.tensor.matmul(
                                    ps[:],
                                    x_slice(sc, ko, mt_i),
                                    w_slice(ko, half),
                                    start=ko == 2,
                                    stop=ko == KO - 1,
                                )
                            evict_half(ps, ot, half)
                            dr_half(sc, mt_i, ot, half)
                    if not final:
                        nc.scalar.dma_start(y_v[:, mt, :], ot[:])
                if sc == n_sc - 2:
                    # bf16 ko0-1 slice for the final (pure-bf16) m-tile
                    xf = xpool.tile([P, 2, P], bf16, tag="xf")
                    nc.sync.dma_start(xf[:], xT_v[:, 0:2, M - P : M])

    nc.compile()
    return nc


def _get_nc():
    global _NC_CACHE
    if _NC_CACHE is None:
        _NC_CACHE = _build_nc()
    return _NC_CACHE


def kernel(x, W, b, A, B):
    global LAST_RESULT
    x = np.ascontiguousarray(np.asarray(x, dtype=np.float32))
    W = np.asarray(W, dtype=np.float32)
    b = np.asarray(b, dtype=np.float32)
    A = np.asarray(A, dtype=np.float32)
    B = np.asarray(B, dtype=np.float32)
    assert x.shape == (4, 4096, D) and W.shape == (O, D)
    assert b.shape == (O,) and A.shape[1] == D and B.shape[0] == O

    # Fold the LoRA update into the weight: x@W^T + s*(x@A^T)@B^T = x@(W + s*B@A)^T
    Weff = (
        W.astype(np.float64) + SCALING * (B.astype(np.float64) @ A.astype(np.float64))
    ).astype(np.float32)
    WeffT = np.ascontiguousarray(Weff.T)  # [D, O] f32
    wT_bf = WeffT.astype(ml_dtypes.bfloat16)
    w8_q = np.ascontiguousarray(SW * WeffT[: 2 * P]).astype(ml_dtypes.float8_e4m3)
    bias_rep = np.ascontiguousarray(np.broadcast_to(b[None, :], (P, O)))

    xb = x.reshape(M_TOTAL, D).astype(ml_dtypes.bfloat16)
    x8_full = (SX * x.reshape(M_TOTAL, D)[:, : 2 * P]).astype(ml_dtypes.float8_e4m3)
    in_maps = []
    for c in range(N_CORES):
        xTc = np.ascontiguousarray(xb[c * M : (c + 1) * M].T)  # [D, M] bf16
        x8c = np.ascontiguousarray(x8_full[c * M : (c + 1) * M].T)  # [256, M] fp8
        in_maps.append(
            {"xT": xTc, "wT": wT_bf, "x8": x8c, "w8": w8_q, "bias": bias_rep}
        )

    nc = _get_nc()
    res = run_bass_kernel_spmd(
        nc, in_maps, core_ids=list(range(N_CORES)), trace=TRACE
    )
    LAST_RESULT = res

    out = np.concatenate([res.results[c]["y"] for c in range(N_CORES)], axis=0)
    return out.astype(np.float32).reshape(x.shape[0], x.shape[1], O)
